# revision 1
# baseline (speedup 1.0000x reference)
"""DEDICOM decoder forward on 8 Trainium2 NeuronCores.

Math per relation k (k=0..7):
    M_k = diag(d_k) @ G @ diag(d_k)                  (64x64, host-precomputed)
    out[k, n] = sigmoid( (row_n @ M_k) . col_n )

Device algorithm (data-parallel over N across 8 cores; per core SHARD=62500
samples padded to 512*128=65536; sample s lives at (p=s//512, t=s%512)):

  Stage 1 (PE): per tile t (128 samples) and k-pair p in 0..3:
      Y^T[(kappa,j), n] = sum_i Mpair_p[i, (kappa,j)] * rowT[i, n]
    i.e. matmul(lhsT=Mquad[:,p,:] [64,128], rhs=rowT_g[:,t,:] [64,128])
    -> PSUM f32 [128, 128], four pairs packed in one [128,512] bank slice.

  Col-multiply U^T = Y^T * colT2 (colT duplicated on both partition halves),
  split across three engines per 2-tile block:
      ACT   : bridge pairs 0-1  PSUM f32 -> SBUF bf16
      DVE   : bf16 mult pairs 0-1; f32 PSUM-direct mult pair 2
      Pool  : f32 PSUM-direct mult pair 3

  Stage 2 (PE): reduce over j=64 per kappa via constant selection matrix:
      matmul(lhsT=U^T[:,b,pair,:] [128,128], rhs=sel [128,2])
    -> rec PSUM [128, 2] slices of a [128,64,8] bank (k = 2*pair+kappa).

  ACT: sigmoid per 64-tile group [128,64,8] -> SBUF f32; DMA out.
"""

import sys

sys.path.insert(0, "/opt/trn_rl_repo")

import numpy as np
import ml_dtypes

import concourse.bass as bass
import concourse.bacc as bacc
import concourse.mybir as mybir
from concourse import tile
from concourse.bass_utils import run_bass_kernel_spmd

N, D, R = 500000, 64, 8
NCORES = 8
SHARD = N // NCORES            # 62500
TPP = 512                      # samples per partition; 512*128 = 65536 >= 62500
SHARD_PAD = TPP * 128
W = 64                         # tiles per group (rec PSUM bank = [128,64,8] f32)
NGROUPS = TPP // W             # 8
BF16 = mybir.dt.bfloat16
F32 = mybir.dt.float32

_CACHE: dict = {}


def _build_program():
    if "nc" in _CACHE:
        return _CACHE["nc"]

    nc = bacc.Bacc(
        "TRN2", target_bir_lowering=False, debug=False, num_devices=NCORES
    )

    rowT_d = nc.dram_tensor("rowt", [D, TPP, 128], BF16, kind="ExternalInput")
    colT_d = nc.dram_tensor("colt", [D, TPP, 128], BF16, kind="ExternalInput")
    mq_d = nc.dram_tensor("mquad", [128, 2 * 128], BF16, kind="ExternalInput")
    sel_d = nc.dram_tensor("sel", [128, 2], BF16, kind="ExternalInput")
    out_d = nc.dram_tensor("out", [SHARD_PAD, R], F32, kind="ExternalOutput")

    out_v = out_d.ap().rearrange("(p t) k -> p t k", p=128)

    MULT = mybir.AluOpType.mult

    with tile.TileContext(nc) as tc:
        with (
            tc.tile_pool(name="const", bufs=1) as cpool,
            tc.tile_pool(name="io", bufs=2) as iopool,
            tc.tile_pool(name="work", bufs=4) as wpool,
            tc.tile_pool(name="psum_y0", bufs=2, space="PSUM") as py0_pool,
            tc.tile_pool(name="psum_y1", bufs=2, space="PSUM") as py1_pool,
            tc.tile_pool(name="psum_r", bufs=2, space="PSUM") as pr_pool,
        ):
            # mquad2[64h+i, q, (kappa,j)] = M_{2*(2q+h)+kappa}[i, j]
            mquad = cpool.tile([128, 2, 128], BF16, tag="mquad")
            sel = cpool.tile([128, 2], BF16, tag="sel")
            nc.sync.dma_start(
                mquad[:].rearrange("d q j -> d (q j)"), mq_d.ap()
            )
            nc.sync.dma_start(sel[:], sel_d.ap())

            for g in range(NGROUPS):
                t0 = g * W
                rowT_g = iopool.tile([128, W, 128], BF16, tag="rowT_g")
                colT2_g = iopool.tile([128, W, 128], BF16, tag="colT2_g")
                # rowT/colT duplicated onto both partition halves (PE row tiling)
                nc.sync.dma_start(
                    rowT_g[0:64, :, :], rowT_d.ap()[:, t0 : t0 + W, :]
                )
                nc.sync.dma_start(
                    rowT_g[64:128, :, :], rowT_d.ap()[:, t0 : t0 + W, :]
                )
                nc.sync.dma_start(
                    colT2_g[0:64, :, :], colT_d.ap()[:, t0 : t0 + W, :]
                )
                nc.sync.dma_start(
                    colT2_g[64:128, :, :], colT_d.ap()[:, t0 : t0 + W, :]
                )

                rec_ps = pr_pool.tile([128, W, R], F32, tag="rec")

                for b0 in range(0, W, 2):
                    # stage 1, row-tiled: h=0/1 halves run concurrently on
                    # the PE; each half owns its own PSUM bank (y0 / y1).
                    # y_h[:, b, q, :] holds pair p = 2q+h.
                    y0 = py0_pool.tile([128, 2, 2, 128], F32, tag="y0")
                    y1 = py1_pool.tile([128, 2, 2, 128], F32, tag="y1")
                    for q in range(2):
                        for b in range(2):
                            t = b0 + b
                            for h, yh in ((0, y0), (1, y1)):
                                nc.tensor.matmul(
                                    yh[:, b, q, :],
                                    mquad[64 * h : 64 * h + 64, q, :],
                                    rowT_g[64 * h : 64 * h + 64, t, :],
                                )

                    colb = colT2_g[:, b0 : b0 + 2, :]
                    # ut free index m: 0=pair0, 1=pair2, 2=pair1, 3=pair3
                    ut = wpool.tile([128, 2, 4, 128], BF16, tag="ut")

                    # ACT: bridge pairs 1,3 (=y1) to bf16
                    ybf = wpool.tile([128, 2, 2, 128], BF16, tag="ybf")
                    nc.scalar.copy(ybf[:], y1[:])
                    # DVE: f32 PSUM-direct mult pairs 0,2 (=y0)
                    nc.vector.tensor_tensor(
                        out=ut[:, :, 0:2, :],
                        in0=y0[:],
                        in1=colb.unsqueeze(2).broadcast_to([128, 2, 2, 128]),
                        op=MULT,
                    )
                    # DVE: bf16 mult pair 1
                    nc.vector.tensor_tensor(
                        out=ut[:, :, 2, :],
                        in0=ybf[:, :, 0, :],
                        in1=colb,
                        op=MULT,
                    )
                    # Pool: bf16 mult pair 3 (SBUF only)
                    nc.gpsimd.tensor_tensor(
                        out=ut[:, :, 3, :],
                        in0=ybf[:, :, 1, :],
                        in1=colb,
                        op=MULT,
                    )

                    # stage 2: PE reduce over j via selection matrix
                    for b in range(2):
                        t = b0 + b
                        for m, p in ((0, 0), (1, 2), (2, 1), (3, 3)):
                            nc.tensor.matmul(
                                rec_ps[:, t, 2 * p : 2 * p + 2],
                                ut[:, b, m, :],
                                sel[:],
                            )

                sig_g = wpool.tile([128, W, R], F32, tag="sig")
                nc.scalar.activation(
                    sig_g[:],
                    rec_ps[:],
                    mybir.ActivationFunctionType.Sigmoid,
                )
                nc.sync.dma_start(out_v[:, t0 : t0 + W, :], sig_g[:])

    nc.compile()
    _CACHE["nc"] = nc
    return nc


def _prep_inputs(inputs_row, inputs_col, global_interaction, local_variation):
    d = np.asarray(local_variation, np.float32)
    g = np.asarray(global_interaction, np.float32)
    # Mquad[i, p, (kappa, j)] = M_{2p+kappa}[i, j] = d[k,i]*G[i,j]*d[k,j]
    mk = np.einsum("ki,ij,kj->kij", d, g, d)            # [8, 64, 64]
    # mq2[64h+i, (q, kappa, j)] = M_{2*(2q+h)+kappa}[i, j]
    mq2 = np.zeros((128, 2, 2, D), np.float32)
    for h in range(2):
        for q in range(2):
            for kap in range(2):
                mq2[64 * h : 64 * h + 64, q, kap, :] = mk[2 * (2 * q + h) + kap]
    mquad = mq2.reshape(128, 2 * 128).astype(ml_dtypes.bfloat16)
    sel = np.zeros((128, 2), np.float32)
    sel[0:64, 0] = 1.0
    sel[64:128, 1] = 1.0
    sel = sel.astype(ml_dtypes.bfloat16)

    pad = SHARD_PAD - SHARD
    in_maps = []
    for c in range(NCORES):
        sl = slice(c * SHARD, (c + 1) * SHARD)
        rr = np.concatenate(
            [np.asarray(inputs_row[sl], np.float32), np.zeros((pad, D), np.float32)]
        ).astype(ml_dtypes.bfloat16)
        cc = np.concatenate(
            [np.asarray(inputs_col[sl], np.float32), np.zeros((pad, D), np.float32)]
        ).astype(ml_dtypes.bfloat16)
        rowt = np.ascontiguousarray(rr.reshape(128, TPP, D).transpose(2, 1, 0))
        colt = np.ascontiguousarray(cc.reshape(128, TPP, D).transpose(2, 1, 0))
        in_maps.append(
            {"rowt": rowt, "colt": colt, "mquad": mquad, "sel": sel}
        )
    return in_maps


def kernel(inputs_row, inputs_col, global_interaction, local_variation):
    nc = _build_program()
    in_maps = _prep_inputs(
        inputs_row, inputs_col, global_interaction, local_variation
    )
    res = run_bass_kernel_spmd(nc, in_maps, list(range(NCORES)))
    outs = [res.results[c]["out"][:SHARD] for c in range(NCORES)]
    full = np.concatenate(outs, axis=0)  # [N, 8] f32
    return np.ascontiguousarray(full.T)  # [8, N]


if __name__ == "__main__":
    rng = np.random.default_rng(0)
    inputs = {
        "inputs_row": rng.standard_normal((N, D), dtype=np.float32),
        "inputs_col": rng.standard_normal((N, D), dtype=np.float32),
        "global_interaction": rng.uniform(-0.2, 0.2, (D, D)).astype(np.float32),
        "local_variation": rng.uniform(-0.3, 0.3, (R, D)).astype(np.float32),
    }
    out = kernel(**inputs)
    print("out", out.shape, out.dtype, out[:, :3])



# revision 7
# speedup vs baseline: 1.2124x; 1.2124x over previous
"""DEDICOM decoder forward on 8 Trainium2 NeuronCores.

Math per relation k (k=0..7):
    M_k = diag(d_k) @ G @ diag(d_k)                  (64x64, host-precomputed)
    out[k, n] = sigmoid( (row_n @ M_k) . col_n )

Device algorithm (data-parallel over N across 8 cores; per core 62500
samples padded to 123*512=62976; block B = 512 samples, half h=s//256):

  Stage 1 (PE, row-tiled): per (block, half h, pair p in 0..3):
      y[(kappa,j), s] = sum_i Mpair_p[i, (kappa,j)] * rowT[i, s]
    matmul(lhsT=mq4[64h:64h+64, p, :] [64,128], rhs=rowc[64h:64h+64, b, :]
    [64,256]) -> PSUM f32 [128, 256]; pairs packed 2-per-bank
    ([128, 2, 256] f32 = one 2KB bank); h=0/1 run concurrently on the
    two 64-row PE tiles.

  Col-multiply U = Y * colT (colT kappa-duplicated on both partition
  halves, host-prepped), split across engines per block:
      DVE  : h0 banks, f32 PSUM-direct tensor_tensor -> SBUF bf16
      ACT  : h1 banks, bridge PSUM f32 -> SBUF bf16
      Pool : h1 pairs 0-1 bf16 mult (scalar_tensor_tensor)
      DVE  : h1 pairs 2-3 bf16 mult (scalar_tensor_tensor, 4x mode)

  Stage 2 (PE): reduce over j via constant selection matrix, ut chunk
  as stationary weights (FWL):
      matmul(lhsT=ut[:, p, 128c:128c+128] [128,128], rhs=sel [128,2])
    -> rec PSUM [128, 2] slices of a [128, 64, 8] bank (k = 2*pair+kappa).

  ACT: sigmoid per chunk [128, 64, 8] -> SBUF f32; DMA out.
"""

import sys

sys.path.insert(0, "/opt/trn_rl_repo")

import numpy as np
import ml_dtypes

import concourse.bass as bass
import concourse.bacc as bacc
import concourse.mybir as mybir
from concourse import tile
from concourse.bass_utils import run_bass_kernel_spmd

N, D, R = 500000, 64, 8
NCORES = 8
SHARD = N // NCORES            # 62500
BLK = 512                      # samples per block (256 per PE half)
NBLK = (SHARD + BLK - 1) // BLK  # 123
SHARD_PAD = NBLK * BLK         # 62976
CHUNK = 16                     # blocks per DMA chunk / rec bank
NCHUNK = (NBLK + CHUNK - 1) // CHUNK
BF16 = mybir.dt.bfloat16
F32 = mybir.dt.float32

_CACHE: dict = {}


def _build_program():
    if "nc" in _CACHE:
        return _CACHE["nc"]

    nc = bacc.Bacc(
        "TRN2", target_bir_lowering=False, debug=False, num_devices=NCORES
    )

    rowc_d = nc.dram_tensor("rowc", [128, NBLK, 256], BF16, kind="ExternalInput")
    colc_d = nc.dram_tensor("colc", [128, NBLK, 2, 256], BF16, kind="ExternalInput")
    mq_d = nc.dram_tensor("mq4", [128, 4 * 128], BF16, kind="ExternalInput")
    sel_d = nc.dram_tensor("sel", [128, 2], BF16, kind="ExternalInput")
    out_d = nc.dram_tensor("out", [128, NBLK, 4, R], F32, kind="ExternalOutput")

    MULT = mybir.AluOpType.mult

    with tile.TileContext(nc) as tc:
        with (
            tc.tile_pool(name="const", bufs=1) as cpool,
            tc.tile_pool(name="iorow", bufs=3) as rowpool,
            tc.tile_pool(name="iocol", bufs=3) as colpool,
            tc.tile_pool(name="ut0", bufs=3) as ut0pool,
            tc.tile_pool(name="ut1", bufs=3) as ut1pool,
            tc.tile_pool(name="ybf", bufs=3) as ybfpool,
            tc.tile_pool(name="sig", bufs=2) as sigpool,
            tc.tile_pool(name="py0", bufs=2, space="PSUM") as py0,
            tc.tile_pool(name="py1", bufs=1, space="PSUM") as py1,
            tc.tile_pool(name="psum_r", bufs=2, space="PSUM") as pr_pool,
        ):
            # mq4[64h+i, p, 64kap+j] = M_{2p+kap}[i, j]  (same for h=0/1)
            mq4 = cpool.tile([128, 4, 128], BF16, tag="mq4")
            sel = cpool.tile([128, 2], BF16, tag="sel")
            nc.sync.dma_start(mq4[:].rearrange("d p j -> d (p j)"), mq_d.ap())
            nc.sync.dma_start(sel[:], sel_d.ap())

            ypools = (py0, py1)

            for ci in range(NCHUNK):
                b0 = ci * CHUNK
                nb = min(CHUNK, NBLK - b0)
                rowt = rowpool.tile([128, CHUNK, 256], BF16, tag="rowt")
                colt = colpool.tile([128, CHUNK, 2, 256], BF16, tag="colt")
                nc.sync.dma_start(
                    rowt[:, 0:nb, :], rowc_d.ap()[:, b0 : b0 + nb, :]
                )
                nc.sync.dma_start(
                    colt[:, 0:nb, :, :], colc_d.ap()[:, b0 : b0 + nb, :, :]
                )

                rec = pr_pool.tile([128, 4 * CHUNK, R], F32, tag="rec")

                for b in range(nb):
                    # ---- stage 1: 8 matmuls, h halves run concurrently
                    ys = []
                    for h in range(2):
                        yduo = []
                        for duo in range(2):
                            y = ypools[h].tile(
                                [128, 2, 256], F32, tag=f"y{h}{duo}"
                            )
                            yduo.append(y)
                        ys.append(yduo)
                    for duo in range(2):
                        for q in range(2):
                            p = 2 * duo + q
                            for h in range(2):
                                nc.tensor.matmul(
                                    ys[h][duo][:, q, :],
                                    mq4[64 * h : 64 * h + 64, p, :],
                                    rowt[64 * h : 64 * h + 64, b, :],
                                )

                    # ---- col multiply: U = Y * colT  (bf16 out)
                    ut0 = ut0pool.tile([128, 4, 256], BF16, tag="ut0")
                    ut1 = ut1pool.tile([128, 4, 256], BF16, tag="ut1")
                    col0 = colt[:, b, 0, :].unsqueeze(1)
                    col1 = colt[:, b, 1, :].unsqueeze(1)

                    # DVE: h0 both duos, PSUM f32 direct
                    for duo in range(2):
                        nc.vector.tensor_tensor(
                            out=ut0[:, 2 * duo : 2 * duo + 2, :],
                            in0=ys[0][duo][:],
                            in1=col0.broadcast_to([128, 2, 256]),
                            op=MULT,
                        )
                    # ACT: bridge h1 banks PSUM f32 -> SBUF bf16
                    ybf = ybfpool.tile([128, 4, 256], BF16, tag="ybf")
                    for duo in range(2):
                        nc.scalar.copy(
                            ybf[:, 2 * duo : 2 * duo + 2, :], ys[1][duo][:]
                        )
                    # Pool: h1 pairs 0-1 bf16 mult
                    nc.gpsimd.tensor_tensor(
                        out=ut1[:, 0:2, :],
                        in0=ybf[:, 0:2, :],
                        in1=col1.broadcast_to([128, 2, 256]),
                        op=MULT,
                    )
                    # DVE: h1 pairs 2-3 bf16 mult (4x mode)
                    nc.vector.scalar_tensor_tensor(
                        out=ut1[:, 2:4, :],
                        in0=ybf[:, 2:4, :],
                        scalar=1.0,
                        in1=col1.broadcast_to([128, 2, 256]),
                        op0=MULT,
                        op1=MULT,
                    )

                    # ---- stage 2: PE reduce over j, ut chunks as weights
                    for h, ut in ((0, ut0), (1, ut1)):
                        for c in range(2):
                            slot = 4 * b + 2 * h + c
                            for p in range(4):
                                nc.tensor.matmul(
                                    rec[:, slot, 2 * p : 2 * p + 2],
                                    ut[:, p, 128 * c : 128 * c + 128],
                                    sel[:],
                                )

                sig = sigpool.tile([128, 4 * CHUNK, R], F32, tag="sig")
                nc.scalar.activation(
                    sig[:, 0 : 4 * nb, :],
                    rec[:, 0 : 4 * nb, :],
                    mybir.ActivationFunctionType.Sigmoid,
                )
                nc.sync.dma_start(
                    out_d.ap()[:, b0 : b0 + nb, :, :],
                    sig[:, 0 : 4 * nb, :].rearrange(
                        "n (b hc) k -> n b hc k", hc=4
                    ),
                )

    nc.compile()
    _CACHE["nc"] = nc
    return nc


def _prep_inputs(inputs_row, inputs_col, global_interaction, local_variation):
    d = np.asarray(local_variation, np.float32)
    g = np.asarray(global_interaction, np.float32)
    # mk[k, i, j] = d[k,i]*G[i,j]*d[k,j]
    mk = np.einsum("ki,ij,kj->kij", d, g, d)            # [8, 64, 64]
    # mq4[64h+i, p, 64kap+j] = mk[2p+kap, i, j]
    m4 = mk.reshape(4, 2, D, D).transpose(2, 0, 1, 3).reshape(D, 4, 2 * D)
    mq4 = np.concatenate([m4, m4], axis=0).reshape(128, 4 * 128)
    mq4 = np.ascontiguousarray(mq4).astype(ml_dtypes.bfloat16)

    sel = np.zeros((128, 2), np.float32)
    sel[0:64, 0] = 1.0
    sel[64:128, 1] = 1.0
    sel = sel.astype(ml_dtypes.bfloat16)

    row_f = np.asarray(inputs_row, np.float32)
    col_f = np.asarray(inputs_col, np.float32)
    pad = SHARD_PAD - SHARD
    in_maps = []
    for cidx in range(NCORES):
        sl = slice(cidx * SHARD, (cidx + 1) * SHARD)
        rr = np.concatenate(
            [row_f[sl], np.zeros((pad, D), np.float32)]
        ).astype(ml_dtypes.bfloat16)
        cc = np.concatenate(
            [col_f[sl], np.zeros((pad, D), np.float32)]
        ).astype(ml_dtypes.bfloat16)
        # rowc[64h+i, B, s] = row[B*512 + 256h + s, i]
        r4 = rr.reshape(NBLK, 2, 256, D).transpose(1, 3, 0, 2)  # [h, i, B, s]
        rowc = np.ascontiguousarray(r4.reshape(128, NBLK, 256))
        # colc[64kap+j, B, h, s] = col[B*512 + 256h + s, j]
        c4 = cc.reshape(NBLK, 2, 256, D).transpose(3, 0, 1, 2)  # [j, B, h, s]
        colc = np.ascontiguousarray(
            np.concatenate([c4, c4], axis=0)
        )  # [128, NBLK, 2, 256]
        in_maps.append(
            {"rowc": rowc, "colc": colc, "mq4": mq4, "sel": sel}
        )
    return in_maps


def kernel(inputs_row, inputs_col, global_interaction, local_variation):
    nc = _build_program()
    in_maps = _prep_inputs(
        inputs_row, inputs_col, global_interaction, local_variation
    )
    res = run_bass_kernel_spmd(nc, in_maps, list(range(NCORES)))
    outs = []
    for c in range(NCORES):
        o = res.results[c]["out"]                   # [128, NBLK, 4, 8]
        # sample = B*512 + hc*128 + p'
        o = o.transpose(1, 2, 0, 3).reshape(SHARD_PAD, R)
        outs.append(o[:SHARD])
    full = np.concatenate(outs, axis=0)             # [N, 8]
    return np.ascontiguousarray(full.T)             # [8, N]


if __name__ == "__main__":
    rng = np.random.default_rng(0)
    inputs = {
        "inputs_row": rng.standard_normal((N, D), dtype=np.float32),
        "inputs_col": rng.standard_normal((N, D), dtype=np.float32),
        "global_interaction": rng.uniform(-0.2, 0.2, (D, D)).astype(np.float32),
        "local_variation": rng.uniform(-0.3, 0.3, (R, D)).astype(np.float32),
    }
    out = kernel(**inputs)
    print("out", out.shape, out.dtype, out[:, :3])
